# revision 6
# baseline (speedup 1.0000x reference)
"""MoE all-to-all token dispatch kernel for 8 Trainium2 NeuronCores.

Problem: out[d, t*K+k, :] = x[t, :] if expert_mapping[expert_indices[t, k]] == d
else 0, with B=4, S=4096, H=512, K=2, 64 experts, 8 devices.

Strategy: the output's leading device axis is sharded across the 8 cores —
core d produces out[d] = [T*K, H].  Only ~1/8 of each core's output rows are
nonzero (each (t, k) slot is owned by exactly one device), so instead of
writing the dense 64 MiB slab, each core gathers just its owned token rows
from HBM into SBUF (dma_gather) and scatter-adds them into the owned slots of
the output (dma_scatter_add).  The output DRAM buffer is pre-zeroed by the
runtime (run_bass_kernel_spmd zero-fills/donates ExternalOutput buffers), so
untouched rows are already correct.

Routing metadata (which rows each core owns) is computed on the host from
expert_indices/expert_mapping and passed per-core as int16 index tensors.
Per-core counts are padded to a common multiple-of-128 maxn with all-valid
indices: padded gather slots read a zero row appended to xin (index T), and
padded scatter slots add those zeros to out row 0 — a no-op.  This keeps the
instruction stream fully static (one NEFF for all 8 cores, no runtime count
registers).
"""

import numpy as np

B, S, H, K = 4, 4096, 512, 2
T = B * S          # 16384 tokens
TK = T * K         # 32768 output rows per device
D = 8              # devices / NeuronCores
E = 64             # experts

ZROW = T           # index of the appended all-zero row in xin

# Set by test harness to collect an NTFF profile; kernel() stores the
# measured exec time (ns) here after each traced run.
TRACE = False
LAST_EXEC_NS = None
LAST_RESULTS = None

_CACHE = {}


def _wrap_idxs16(vals: np.ndarray, maxn: int, pad: int) -> np.ndarray:
    """SWDGE wrapped int16 layout: element i at [i % 16, i // 16], `pad`
    tail, replicated across the 8 partition groups (128 partitions)."""
    arr = np.full(maxn, pad, np.int16)
    arr[: len(vals)] = vals.astype(np.int16)
    w = arr.reshape(maxn // 16, 16).T          # [16, maxn/16]
    return np.ascontiguousarray(np.tile(w, (8, 1)))  # [128, maxn/16]


def _build_module(maxn: int):
    import concourse.bacc as bacc
    import concourse.mybir as mybir
    from concourse.library_config import mlp

    nb = maxn // 128

    nc = bacc.Bacc("TRN2", debug=False)
    xin = nc.dram_tensor("xin", [T + 1, H], mybir.dt.float32,
                         kind="ExternalInput")
    sidx = nc.dram_tensor("sidx", [128, maxn // 16], mybir.dt.int16,
                          kind="ExternalInput")
    didx = nc.dram_tensor("didx", [128, maxn // 16], mybir.dt.int16,
                          kind="ExternalInput")
    out = nc.dram_tensor("out", [TK, H], mybir.dt.float32,
                         kind="ExternalOutput")

    with (
        nc.Block() as block,
        nc.sbuf_tensor("data", [128, nb, H], mybir.dt.float32) as data,
        nc.sbuf_tensor("sidx_sb", [128, maxn // 16], mybir.dt.int16) as sidx_sb,
        nc.sbuf_tensor("didx_sb", [128, maxn // 16], mybir.dt.int16) as didx_sb,
        nc.semaphore("io") as io,
        nc.semaphore("gsem") as gsem,
        nc.semaphore("ssem") as ssem,
    ):
        @block.gpsimd
        def _(gpsimd):
            gpsimd.load_library(mlp)
            gpsimd.dma_start(sidx_sb[:], sidx[:]).then_inc(io, 16)
            gpsimd.dma_start(didx_sb[:], didx[:]).then_inc(io, 16)
            gpsimd.wait_ge(io, 32)
            gpsimd.dma_gather(data[:], xin[:], sidx_sb[:], maxn, maxn, H,
                              single_packet=False).then_inc(gsem, 16)
            gpsimd.wait_ge(gsem, 16)
            gpsimd.dma_scatter_add(out[:], data[:], didx_sb[:], maxn, maxn, H,
                                   single_packet=False).then_inc(ssem, 16)
            gpsimd.wait_ge(ssem, 16)

    nc.compile()
    return nc


def kernel(input_tensor, expert_indices, expert_mapping):
    global LAST_EXEC_NS, LAST_RESULTS
    from concourse.bass_utils import run_bass_kernel_spmd

    x = np.zeros((T + 1, H), dtype=np.float32)
    x[:T] = np.asarray(input_tensor, dtype=np.float32).reshape(T, H)
    idx = np.asarray(expert_indices, dtype=np.int32).reshape(-1)
    emap = np.asarray(expert_mapping, dtype=np.int32)
    owner = emap[idx]                                  # [T*K], slot r = t*K+k

    dsts = [np.nonzero(owner == d)[0] for d in range(D)]
    maxn = -(-max(len(v) for v in dsts) // 128) * 128

    if maxn not in _CACHE:
        _CACHE[maxn] = _build_module(maxn)
    nc = _CACHE[maxn]

    in_maps = []
    for d in range(D):
        dst = dsts[d]
        src = dst // K
        in_maps.append({
            "xin": x,
            "sidx": _wrap_idxs16(src, maxn, pad=ZROW),
            "didx": _wrap_idxs16(dst, maxn, pad=0),
        })

    res = run_bass_kernel_spmd(nc, in_maps, list(range(D)), trace=TRACE)
    if TRACE:
        LAST_EXEC_NS = res.exec_time_ns
        LAST_RESULTS = res
    return np.stack([res.results[d]["out"] for d in range(D)], axis=0)


# revision 10
# speedup vs baseline: 1.1885x; 1.1885x over previous
"""MoE all-to-all token dispatch kernel for 8 Trainium2 NeuronCores.

Problem: out[d, t*K+k, :] = x[t, :] if expert_mapping[expert_indices[t, k]] == d
else 0, with B=4, S=4096, H=512, K=2, 64 experts, 8 devices.

Strategy: the output's leading device axis is sharded across the 8 cores —
core d produces out[d] = [T*K, H].  Only ~1/8 of each core's output rows are
nonzero (each (t, k) slot is owned by exactly one device), so instead of
writing the dense 64 MiB slab, each core gathers just its owned token rows
from HBM into SBUF (dma_gather) and scatter-adds them into the owned slots of
the output (dma_scatter_add).  The output DRAM buffer is pre-zeroed by the
runtime (run_bass_kernel_spmd zero-fills/donates ExternalOutput buffers), so
untouched rows are already correct.

Routing metadata (which rows each core owns) is computed on the host from
expert_indices/expert_mapping and passed per-core as int16 index tensors.
Per-core counts are padded to a common multiple-of-128 maxn with all-valid
indices: padded gather slots read a zero row appended to xin (index T), and
padded scatter slots add those zeros to out row 0 — a no-op.  This keeps the
instruction stream fully static (one NEFF for all 8 cores, no runtime count
registers).
"""

import numpy as np

B, S, H, K = 4, 4096, 512, 2
T = B * S          # 16384 tokens
TK = T * K         # 32768 output rows per device
D = 8              # devices / NeuronCores
E = 64             # experts

ZROW = T           # index of the appended all-zero row in xin

# Set by test harness to collect an NTFF profile; kernel() stores the
# measured exec time (ns) here after each traced run.
TRACE = False
LAST_EXEC_NS = None
LAST_RESULTS = None

_CACHE = {}


def _wrap_idxs16(vals: np.ndarray, maxn: int, pad: int) -> np.ndarray:
    """SWDGE wrapped int16 layout: element i at [i % 16, i // 16], `pad`
    tail, replicated across the 8 partition groups (128 partitions)."""
    arr = np.full(maxn, pad, np.int16)
    arr[: len(vals)] = vals.astype(np.int16)
    w = arr.reshape(maxn // 16, 16).T          # [16, maxn/16]
    return np.ascontiguousarray(np.tile(w, (8, 1)))  # [128, maxn/16]


CH = 512           # slots per chunk (multiple of 128)


def _build_module(maxn: int):
    import concourse.bacc as bacc
    import concourse.mybir as mybir
    from concourse.library_config import mlp
    from contextlib import ExitStack

    assert maxn % CH == 0
    nb = maxn // 128
    nch = maxn // CH
    nbc = CH // 128        # data columns per chunk
    wc = CH // 16          # wrapped-idx columns per chunk

    nc = bacc.Bacc("TRN2", debug=False, num_swdge_queues=4)
    xin = nc.dram_tensor("xin", [T + 1, H], mybir.dt.float32,
                         kind="ExternalInput")
    sidx = nc.dram_tensor("sidx", [128, maxn // 16], mybir.dt.int16,
                          kind="ExternalInput")
    didx = nc.dram_tensor("didx", [128, maxn // 16], mybir.dt.int16,
                          kind="ExternalInput")
    out = nc.dram_tensor("out", [TK, H], mybir.dt.float32,
                         kind="ExternalOutput")

    with (
        nc.Block() as block,
        nc.sbuf_tensor("data", [128, nb, H], mybir.dt.float32) as data,
        nc.sbuf_tensor("sidx_sb", [128, maxn // 16], mybir.dt.int16) as sidx_sb,
        nc.sbuf_tensor("didx_sb", [128, maxn // 16], mybir.dt.int16) as didx_sb,
        nc.semaphore("io") as io,
        nc.semaphore("ssem0") as ssem0,
        nc.semaphore("ssem1") as ssem1,
        ExitStack() as stack,
    ):
        gsems = [stack.enter_context(nc.semaphore(f"g{c}"))  # noqa: ANT232
                 for c in range(nch)]

        @block.gpsimd
        def _(gpsimd):
            gpsimd.load_library(mlp)
            gpsimd.dma_start(sidx_sb[:], sidx[:]).then_inc(io, 16)
            gpsimd.dma_start(didx_sb[:], didx[:]).then_inc(io, 16)
            gpsimd.wait_ge(io, 32)
            # Enqueue every gather chunk up front (queues 0/2); the SDMA
            # engines drain them while scatters (queues 1/3) run behind.
            for c in range(nch):
                gpsimd.dma_gather(
                    data[:, c * nbc:(c + 1) * nbc, :], xin[:],
                    sidx_sb[:, c * wc:(c + 1) * wc], CH, CH, H,
                    single_packet=False, queue_num=(c % 2) * 2,
                ).then_inc(gsems[c], 16)
            for c in range(nch):
                gpsimd.wait_ge(gsems[c], 16)
                gpsimd.dma_scatter_add(
                    out[:], data[:, c * nbc:(c + 1) * nbc, :],
                    didx_sb[:, c * wc:(c + 1) * wc], CH, CH, H,
                    single_packet=False, queue_num=(c % 2) * 2 + 1,
                ).then_inc(ssem0 if c % 2 == 0 else ssem1, 16)
            gpsimd.wait_ge(ssem0, 16 * ((nch + 1) // 2))
            gpsimd.wait_ge(ssem1, 16 * (nch // 2))

    nc.compile()
    return nc


def kernel(input_tensor, expert_indices, expert_mapping):
    global LAST_EXEC_NS, LAST_RESULTS
    from concourse.bass_utils import run_bass_kernel_spmd

    x = np.zeros((T + 1, H), dtype=np.float32)
    x[:T] = np.asarray(input_tensor, dtype=np.float32).reshape(T, H)
    idx = np.asarray(expert_indices, dtype=np.int32).reshape(-1)
    emap = np.asarray(expert_mapping, dtype=np.int32)
    owner = emap[idx]                                  # [T*K], slot r = t*K+k

    dsts = [np.nonzero(owner == d)[0] for d in range(D)]
    maxn = -(-max(len(v) for v in dsts) // CH) * CH

    if maxn not in _CACHE:
        _CACHE[maxn] = _build_module(maxn)
    nc = _CACHE[maxn]

    in_maps = []
    for d in range(D):
        dst = dsts[d]
        src = dst // K
        in_maps.append({
            "xin": x,
            "sidx": _wrap_idxs16(src, maxn, pad=ZROW),
            "didx": _wrap_idxs16(dst, maxn, pad=0),
        })

    res = run_bass_kernel_spmd(nc, in_maps, list(range(D)), trace=TRACE)
    if TRACE:
        LAST_EXEC_NS = res.exec_time_ns
        LAST_RESULTS = res
    return np.stack([res.results[d]["out"] for d in range(D)], axis=0)
